# revision 35
# baseline (speedup 1.0000x reference)
"""GCN node classification on 8 Trainium2 NeuronCores (Bass/Tile).

Strategy (dst-sharded graph parallel), v4:
  - Nodes padded to 100352 = 8 * 12544; core c owns dst nodes
    [c*12544, (c+1)*12544)  (98 tiles of 128).
  - Per layer: each core computes xw = g_own @ W on PE; an AllGather makes
    the full [100352, F] bf16 feature table resident on every core's HBM.
  - Self-loop terms are folded into the edge list as explicit self-edges
    (coef = 2*dinv^2), so aggregation is one uniform gather+matmul pass.
  - Edges are bucketed by (dst-tile, 25088-row src window = "run"), sorted
    by src; run lengths are the max over the 8 cores (SPMD-uniform
    schedule), rounded to 16; shorter cores pad with idx 0 and zero rows
    in M. Runs are packed into one index stream per window; dma_gather
    calls of up to 2048 idxs pull source rows (int16 idx, relative to the
    window; the full idx stream is SBUF-resident). Host-precomputed
    selection matrices M[e,d] = coef[e] * (d == dst_local[e]) (bf16) are
    DMA-streamed from HBM, and PE accumulates psum += Y^T @ M (layers
    0-2: output [feat, dst], transposed epilogue) or psum += M^T @ Y
    (layer 3: [dst, feat] for the node-major output).
  - Epilogue per tile (layers 0-2, feat-major): +residual (DVE),
    gelu(agg + bias) in one ACT op with per-partition bias, then the next
    layer's matmul directly from the feat-major activation (no transpose
    needed), DMA into the next collective's input buffer.
"""
import sys

sys.path.insert(0, "/opt/trn_rl_repo")

import numpy as np

import concourse.bass as bass  # noqa: E402
import concourse.tile as tile  # noqa: E402
from concourse import bacc, mybir  # noqa: E402
from concourse.bass_utils import run_bass_kernel_spmd  # noqa: E402

NCORES = 8
F = 128          # feature width (all layers padded to 128)
TILES = 98       # dst tiles per core
OWN = TILES * 128            # 12544 nodes per core
NT = NCORES * OWN            # 100352 padded nodes
HTILES = TILES // 2          # dst tiles per phase (half)
HROWS = HTILES * 128         # 6272 rows per half
NWIN = 4                     # table src windows (2 cores' rows each)
NWINQ = NWIN + 1             # + window 0: self-edges, gathered from cc_in
WIN = 2 * OWN                # 25088 rows per table window (int16-addressable)
GCALL = 4096                 # idxs per dma_gather call
C_OUT = 40
YBUFS = 4
MBUFS = 3
NQUSE = 4
NPHASE = 2                   # dst-tile phases
NSW = NPHASE * NWINQ         # super-windows = (phase, window)


# --------------------------------------------------------------------------
# host-side schedule
# --------------------------------------------------------------------------

class Sched:
    """Shared (core-independent) schedule.

    runs:  list of dicts {q, t, R, s_lo (stream pos within q), first/last}
    calls: list of dicts {q, lo, n, chunks: [ {slot, segs: [
               {run_idx, e_lo, e_hi, m_idx, start, stop} ] } ]}
    MX:    total number of M tiles
    IDXC:  int16 idx columns (sum over calls of n/16)
    """


def make_schedule(R_tw):
    """R_tw: [TILES, NWINQ] run lengths. Super-window sw = phase*NWINQ + w
    covers dst tiles [phase*HTILES, (phase+1)*HTILES). Window 0 is the
    self-edge window (gathers own xw rows from cc_in, collective-free)."""
    sched = Sched()
    sched.runs = []
    qlen = [0] * NSW
    for q in range(NSW):
        p, w = divmod(q, NWINQ)
        pos = 0
        for t in range(p * HTILES, (p + 1) * HTILES):
            R = int(R_tw[t, w])
            if R == 0:
                continue
            sched.runs.append(dict(q=q, t=t, R=R, s_lo=pos, idx=len(sched.runs)))
            pos += R
        if pos % 16:  # idx stream wraps in 16s: pad the window's last run
            pad = 16 - pos % 16
            sched.runs[-1]["R"] += pad
            pos += pad
        qlen[q] = pos

    first_q, last_q = {}, {}
    for r in sched.runs:
        first_q.setdefault(r["t"], r["q"])
        last_q[r["t"]] = r["q"]
    sched.first_q, sched.last_q = first_q, last_q

    # runs of each q sorted by s_lo already
    runs_by_q = [[r for r in sched.runs if r["q"] == q] for q in range(NSW)]

    sched.calls = []
    m_idx = 0
    idxc = 0
    for q in range(NSW):
        rq = runs_by_q[q]
        pos = 0
        ri = 0
        while pos < qlen[q]:
            n = min(GCALL, qlen[q] - pos)
            call = dict(q=q, lo=pos, n=n, idx_off16=idxc, chunks=[])
            idxc += n // 16
            nch = (n + 127) // 128
            for k in range(nch):
                c_lo = pos + k * 128
                c_hi = min(pos + (k + 1) * 128, pos + n)
                segs = []
                # advance ri to first run overlapping c_lo
                while ri < len(rq) and rq[ri]["s_lo"] + rq[ri]["R"] <= c_lo:
                    ri += 1
                rj = ri
                while rj < len(rq) and rq[rj]["s_lo"] < c_hi:
                    r = rq[rj]
                    e_lo = max(r["s_lo"], c_lo) - c_lo
                    e_hi = min(r["s_lo"] + r["R"], c_hi) - c_lo
                    segs.append(dict(
                        run=r, e_lo=e_lo, e_hi=e_hi, m_idx=m_idx,
                        start=(max(r["s_lo"], c_lo) == r["s_lo"]),
                        stop=(min(r["s_lo"] + r["R"], c_hi) == r["s_lo"] + r["R"]),
                    ))
                    m_idx += 1
                    rj += 1
                call["chunks"].append(dict(slot=k, segs=segs))
            sched.calls.append(call)
            pos += n
    sched.MX = m_idx
    sched.IDXC = idxc
    sched.max_nm = max(sum(len(c["segs"]) for c in call["chunks"])
                       for call in sched.calls)
    sched.qlen = qlen
    return sched


def preprocess(x, edge_index, n_real):
    src_r = np.asarray(edge_index[0], dtype=np.int64)
    dst_r = np.asarray(edge_index[1], dtype=np.int64)

    deg = np.bincount(dst_r, minlength=NT).astype(np.float32) + 2.0
    dinv = 1.0 / np.sqrt(deg)

    # fold the self-loop term into the edge list: one self-edge per node
    # with coef = 2*dinv^2 (PyG improved=True self-loop weight).
    allv = np.arange(NT, dtype=np.int64)
    src = np.concatenate([src_r, allv])
    dst = np.concatenate([dst_r, allv])
    coef_a = np.concatenate([
        (dinv[src_r] * dinv[dst_r]).astype(np.float32),
        (2.0 * dinv * dinv).astype(np.float32),
    ])

    core = dst // OWN
    dstl_a = dst - core * OWN
    blk_a = dstl_a >> 7
    dloc_a = (dstl_a & 127).astype(np.int64)
    is_self = np.zeros(src.shape[0], dtype=bool)
    is_self[src_r.shape[0]:] = True
    w_r = np.minimum(src // WIN, NWIN - 1)

    # Per-core permutation of physical dst blocks -> logical tile slots,
    # greedily matching per-window count vectors across cores so the
    # SPMD max-over-cores run lengths carry less padding.
    cnt = np.zeros((NCORES, TILES, NWIN), np.int64)
    np.add.at(cnt, (core[~is_self], blk_a[~is_self], w_r[~is_self]), 1)
    totals = cnt.sum(axis=2)
    remaining = [list(range(TILES)) for _ in range(NCORES)]
    perm = np.zeros((NCORES, TILES), np.int64)
    for slot in range(TILES):
        b0 = max(remaining[0], key=lambda b: totals[0, b])
        remaining[0].remove(b0)
        perm[0, slot] = b0
        vec = cnt[0, b0]
        for c in range(1, NCORES):
            bb = min(remaining[c], key=lambda b: np.abs(cnt[c, b] - vec).sum())
            remaining[c].remove(bb)
            perm[c, slot] = bb
    inv_perm = np.zeros_like(perm)
    for c in range(NCORES):
        inv_perm[c, perm[c]] = np.arange(TILES)
    sched_rows = [(perm[c][:, None] * 128 + np.arange(128)).ravel()
                  for c in range(NCORES)]

    # logical dst tile of each edge
    t_a = inv_perm[core, blk_a]
    # src side: window 0 = self-edges, gathered from cc_in (own xw, static
    # address, no collective dependency; logical row order). Real edges use
    # table windows 1..4 over the AllGather output (logical order per core).
    cs_a = src // OWN
    srcl = src % OWN
    src_log = inv_perm[cs_a, srcl >> 7] * 128 + (srcl & 127)
    w_a = np.where(is_self, 0, 1 + w_r)
    idxrel_a = np.where(is_self, src_log,
                        (cs_a - 2 * w_r) * OWN + src_log).astype(np.int64)
    assert idxrel_a.max() < 32768
    phase_a = (t_a >= HTILES).astype(np.int64)
    q_a = phase_a * NWINQ + w_a

    counts = np.zeros((NCORES, TILES, NWINQ), dtype=np.int64)
    np.add.at(counts, (core, t_a, w_a), 1)
    R_tw = counts.max(axis=0).astype(np.int64)

    sched = make_schedule(R_tw)
    sched.rows_phys = sched_rows

    order = np.lexsort((idxrel_a, t_a, q_a, core))
    src_s = idxrel_a[order]
    core_s = core[order]
    t_s = t_a[order]
    q_s = q_a[order]
    dloc_s = dloc_a[order]
    coef_s = coef_a[order]

    run_pos = {(r["q"], r["t"]): r for r in sched.runs}
    # stream-global base per q
    qbase = np.cumsum([0] + sched.qlen[:-1])

    per_core = []
    for c in range(NCORES):
        sel = core_s == c
        ci, ct, cq = src_s[sel], t_s[sel], q_s[sel]
        cd, cc = dloc_s[sel], coef_s[sel]
        # flat global stream of idx / dloc / coef (padded)
        SL = int(sum(sched.qlen))
        idx_flat = np.zeros(SL, dtype=np.int16)
        dl_flat = np.zeros(SL, dtype=np.int64)
        cf_flat = np.zeros(SL, dtype=np.float32)
        key = cq * TILES + ct
        bounds = np.flatnonzero(np.r_[True, key[1:] != key[:-1], True])
        for bi in range(len(bounds) - 1):
            lo, hi = bounds[bi], bounds[bi + 1]
            r = run_pos[(int(cq[lo]), int(ct[lo]))]
            n = hi - lo
            assert n <= r["R"]
            g0 = qbase[r["q"]] + r["s_lo"]
            idx_flat[g0:g0 + n] = ci[lo:hi]
            dl_flat[g0:g0 + n] = cd[lo:hi]
            cf_flat[g0:g0 + n] = cc[lo:hi]
        # idx wrapped per call
        idx_w = np.zeros((128, sched.IDXC), dtype=np.int16)
        for call in sched.calls:
            g0 = qbase[call["q"]] + call["lo"]
            blk = idx_flat[g0:g0 + call["n"]].reshape(-1, 16).T
            o = call["idx_off16"]
            idx_w[:, o:o + call["n"] // 16] = np.tile(blk, (8, 1))
        # M tiles per segment
        M = np.zeros((sched.MX, 128, 128), dtype=np.float32)
        for call in sched.calls:
            g0 = qbase[call["q"]] + call["lo"]
            for ch in call["chunks"]:
                c_lo = g0 + ch["slot"] * 128
                for s in ch["segs"]:
                    e = np.arange(s["e_lo"], s["e_hi"])
                    gpos = c_lo + e
                    m = M[s["m_idx"]]
                    m[e, dl_flat[gpos]] = cf_flat[gpos]
        M_w = M.transpose(1, 0, 2).reshape(128, sched.MX * 128)
        per_core.append({
            "idx16": np.ascontiguousarray(idx_w),
            "M": np.ascontiguousarray(M_w),
        })

    return per_core, sched


# --------------------------------------------------------------------------
# bass program
# --------------------------------------------------------------------------

def build(sched):
    BDT = mybir.dt.bfloat16
    nc = bacc.Bacc("TRN2", target_bir_lowering=False, debug=False,
                   num_devices=NCORES, num_swdge_queues=NQUSE)

    MX, IDXC = sched.MX, sched.IDXC
    xT_in = nc.dram_tensor("xT", [128, OWN], BDT, kind="ExternalInput")
    idx16_in = nc.dram_tensor("idx16", [128, IDXC], mybir.dt.int16, kind="ExternalInput")
    m_in = nc.dram_tensor("M", [128, MX * 128], BDT, kind="ExternalInput")
    w_in = [nc.dram_tensor(f"W{l}", [128, 128], BDT, kind="ExternalInput")
            for l in range(4)]
    bcol_in = [nc.dram_tensor(f"bc{l}", [128, 1], mybir.dt.float32,
                              kind="ExternalInput") for l in range(3)]
    b3_in = nc.dram_tensor("b3t", [128, 128], mybir.dt.float32, kind="ExternalInput")
    out_dram = nc.dram_tensor("out", [OWN, 128], mybir.dt.float32, kind="ExternalOutput")

    max_call_chunks = max((c["n"] + 127) // 128 for c in sched.calls)
    # M tiles per call
    call_m0 = []
    for call in sched.calls:
        first_seg = call["chunks"][0]["segs"][0]["m_idx"]
        nm = sum(len(ch["segs"]) for ch in call["chunks"])
        call_m0.append((first_seg, nm))
    max_nm = sched.max_nm

    with tile.TileContext(nc) as tc:
        with (
            tc.tile_pool(name="persist", bufs=1) as pers,
            tc.tile_pool(name="ybuf", bufs=YBUFS) as yp,
            tc.tile_pool(name="mbuf", bufs=MBUFS) as mp,
            tc.tile_pool(name="runp", bufs=7, space="PSUM") as rp,
            tc.tile_pool(name="epip", bufs=1, space="PSUM") as ep,
            tc.tile_pool(name="etmp", bufs=4) as et,
            tc.tile_pool(name="dram", bufs=1, space="DRAM") as dp,
        ):
            # ---- persistent SBUF ----
            idx_t = pers.tile([128, IDXC], mybir.dt.int16, tag="idx")
            nc.sync.dma_start(idx_t[:], idx16_in[:])
            xT_t = pers.tile([128, OWN], BDT, tag="xT")
            nc.sync.dma_start(xT_t[:], xT_in[:])
            w_t, bc_t = [], []
            for l in range(4):
                wt = pers.tile([128, 128], BDT, tag=f"w{l}")
                nc.sync.dma_start(wt[:], w_in[l][:])
                w_t.append(wt)
            for l in range(3):
                bt = pers.tile([128, 1], mybir.dt.float32, tag=f"bc{l}")
                nc.sync.dma_start(bt[:], bcol_in[l][:])
                bc_t.append(bt)
            b3_t = pers.tile([128, 128], mybir.dt.float32, tag="b3")
            nc.sync.dma_start(b3_t[:], b3_in[:])
            agg_t = pers.tile([128, TILES * 128], mybir.dt.float32, tag="agg")
            g_t = pers.tile([128, TILES * 128], BDT, tag="g")

            # zero gather slots once (short-count gathers leave stale tails;
            # M zero rows null them unless stale bits are NaN)
            for _ in range(YBUFS):
                yz = yp.tile([128, max_call_chunks, 128], BDT, tag="y")
                nc.vector.memset(yz[:], 0.0)

            # ---- collective buffers ----
            cc_in = [dp.tile([OWN, 128], BDT, tag=f"ccin{l}",
                             name=f"ccin{l}") for l in range(4)]
            cc_out = [dp.tile([NT, 128], BDT, tag=f"ccout{i}",
                              name=f"ccout{i}", addr_space="Shared")
                      for i in range(4)]

            def fire_cc(l):
                nc.gpsimd.collective_compute(
                    "AllGather",
                    mybir.AluOpType.bypass,
                    replica_groups=[list(range(NCORES))],
                    ins=[cc_in[l][:].opt()],
                    outs=[cc_out[l][:].opt()],
                )

            def make_xw(l, t):
                lhsT = (xT_t if l == 0 else g_t)[:, t * 128:(t + 1) * 128]
                pxw = ep.tile([128, 128], mybir.dt.float32, space="PSUM", tag="pxw", bufs=1)
                nc.tensor.matmul(out=pxw[:], lhsT=lhsT, rhs=w_t[l][:],
                                 start=True, stop=True)
                xw_sb = et.tile([128, 128], BDT, tag="xwsb")
                nc.vector.tensor_copy(xw_sb[:], pxw[:])
                nc.sync.dma_start(cc_in[l][t * 128:(t + 1) * 128, :], xw_sb[:])

            # ---- layer 0 pre-phase: xw0 = x @ W0 ----
            for t in range(TILES):
                make_xw(0, t)

            gq = [0]

            def epilogue(l, t):
                agg_sl = agg_t[:, t * 128:(t + 1) * 128]
                g_sl = g_t[:, t * 128:(t + 1) * 128]
                if l == 3:
                    h = et.tile([128, 128], mybir.dt.float32, tag="h")
                    nc.vector.tensor_tensor(out=h[:], in0=agg_sl, in1=b3_t[:],
                                            op=mybir.AluOpType.add)
                    nc.sync.dma_start(out_dram[t * 128:(t + 1) * 128, :], h[:])
                    return
                if l in (1, 2):
                    nc.vector.tensor_tensor(out=agg_sl, in0=agg_sl, in1=g_sl,
                                            op=mybir.AluOpType.add)
                nc.scalar.activation(g_sl, agg_sl,
                                     mybir.ActivationFunctionType.Gelu,
                                     bias=bc_t[l][:])
                make_xw(l + 1, t)

            def do_windows(l, qset):
                """Process the given super-windows; epilogues fire inline."""
                psum_of_run = {}
                for ci, call in enumerate(sched.calls):
                    q, n = call["q"], call["n"]
                    if q not in qset:
                        continue
                    w = q % NWINQ
                    if w == 0:
                        in_rows = cc_in[l][:]
                    else:
                        in_rows = cc_out[l][(w - 1) * WIN:w * WIN, :]
                    nch = (n + 127) // 128
                    o16 = call["idx_off16"]
                    y = yp.tile([128, max_call_chunks, 128], BDT, tag="y")
                    nc.gpsimd.dma_gather(
                        out_ap=y[:, :nch, :],
                        in_ap=in_rows,
                        idxs_ap=idx_t[:, o16:o16 + n // 16],
                        num_idxs=n,
                        num_idxs_reg=n,
                        elem_size=128,
                        single_packet=False,
                        queue_num=gq[0] % NQUSE,
                    )
                    gq[0] += 1
                    m0, nm = call_m0[ci]
                    import os as _os2
                    _nmf = 1 if _os2.environ.get("KMS1") == "1" else nm
                    ms = mp.tile([128, max_nm * 128], BDT, tag="ms")
                    nc.scalar.dma_start(ms[:, :_nmf * 128],
                                        m_in[:, m0 * 128:(m0 + _nmf) * 128])
                    for ch in call["chunks"]:
                        k = ch["slot"]
                        for s in ch["segs"]:
                            r = s["run"]
                            rid = r["idx"]
                            if s["start"]:
                                psum_of_run[rid] = rp.tile(
                                    [128, 128], mybir.dt.float32,
                                    space="PSUM", tag="rp", name="rpt")
                            psum = psum_of_run[rid]
                            mi = s["m_idx"] - m0
                            if l == 3:
                                nc.tensor.matmul(
                                    out=psum[:],
                                    lhsT=ms[:, mi * 128:(mi + 1) * 128],
                                    rhs=y[:, k, :],
                                    start=s["start"], stop=s["stop"])
                            else:
                                nc.tensor.matmul(
                                    out=psum[:],
                                    lhsT=y[:, k, :],
                                    rhs=ms[:, mi * 128:(mi + 1) * 128],
                                    start=s["start"], stop=s["stop"])
                            if s["stop"]:
                                t = r["t"]
                                agg_sl = agg_t[:, t * 128:(t + 1) * 128]
                                if q == sched.first_q[t]:
                                    nc.scalar.copy(agg_sl, psum[:])
                                else:
                                    nc.vector.tensor_tensor(
                                        out=agg_sl, in0=agg_sl, in1=psum[:],
                                        op=mybir.AluOpType.add)
                                del psum_of_run[rid]
                                if q == sched.last_q[t]:
                                    epilogue(l, t)
                assert not psum_of_run

            for l in range(4):
                # self-edge windows first: they read cc_in (own xw) and can
                # run while the AllGather (fired just after) is in flight
                do_windows(l, {0, NWINQ})
                fire_cc(l)
                do_windows(l, {1, 2, 3, 4})
                do_windows(l, {NWINQ + 1, NWINQ + 2, NWINQ + 3, NWINQ + 4})

    nc.compile()
    return nc


# --------------------------------------------------------------------------
# public entry point
# --------------------------------------------------------------------------

def _host_inputs(x, edge_index, Ws, bs):
    import ml_dtypes
    n_real = x.shape[0]
    per_core, sched = preprocess(x, edge_index, n_real)

    xpad = np.zeros((NT, F), dtype=np.float32)
    xpad[:n_real] = np.asarray(x, dtype=np.float32)

    W3p = np.zeros((128, 128), np.float32)
    W3p[:, :C_OUT] = Ws[3]
    Wl = [np.asarray(Ws[0], np.float32), np.asarray(Ws[1], np.float32),
          np.asarray(Ws[2], np.float32), W3p]
    b3p = np.zeros(128, np.float32)
    b3p[:C_OUT] = bs[3]

    in_maps = []
    for c in range(NCORES):
        d = per_core[c]
        m = {
            "xT": xpad[c * OWN:(c + 1) * OWN][sched.rows_phys[c]].T
                  .astype(ml_dtypes.bfloat16),
            "idx16": d["idx16"],
            "M": d["M"].astype(ml_dtypes.bfloat16),
            "b3t": np.tile(b3p, (128, 1)),
        }
        for l in range(4):
            m[f"W{l}"] = Wl[l].astype(ml_dtypes.bfloat16)
        for l in range(3):
            m[f"bc{l}"] = np.asarray(bs[l], np.float32).reshape(128, 1)
        in_maps.append(m)
    return in_maps, sched


def kernel(x, edge_index, W0, b0, W1, b1, W2, b2, W3, b3):
    x = np.asarray(x)
    in_maps, sched = _host_inputs(
        x, np.asarray(edge_index), [W0, W1, W2, W3], [b0, b1, b2, b3])
    nc = build(sched)
    res = run_bass_kernel_spmd(nc, in_maps, list(range(NCORES)))
    outs = []
    for c in range(NCORES):
        o_log = res.results[c]["out"]
        o_phys = np.empty_like(o_log)
        o_phys[sched.rows_phys[c]] = o_log
        outs.append(o_phys)
    full = np.concatenate(outs, axis=0)[:x.shape[0], :C_OUT]
    return full.astype(np.float32)


# revision 37
# speedup vs baseline: 1.3012x; 1.3012x over previous
"""GCN node classification on 8 Trainium2 NeuronCores (Bass/Tile).

Strategy (dst-sharded graph parallel), v6:
  - Nodes padded to 100352 = 8 * 12544; core c owns dst nodes
    [c*12544, (c+1)*12544)  (98 tiles of 128). Per core, physical dst
    blocks are permuted into logical tile slots (greedy per-window count
    matching across cores) to minimise the SPMD max-over-cores padding.
  - Per layer: each core computes xw = g_own @ W on PE (bf16); an
    AllGather makes the full [100352, 128] bf16 table resident in HBM.
  - Self-loop terms are explicit self-edges (coef = 2*dinv^2) in a
    dedicated window that gathers from cc_in (own xw, static address):
    uniform 128/tile (zero padding), and its gathers+matmuls overlap the
    AllGather since they do not depend on it (issued before fire_cc).
  - Edges are bucketed by (phase = dst-tile half, window, dst-tile);
    within a run sorted by src; run lengths are the max over the 8 cores
    (SPMD-uniform schedule); shorter cores pad with idx 0 and zero rows
    in M. dma_gather calls of 4096 idxs pull src rows (int16 idx,
    relative to the window; full idx stream SBUF-resident). Host-built
    selection matrices M[e,d] = coef[e] * (d == dst_local[e]) (bf16) are
    DMA-streamed from HBM on the ACT queue, and PE accumulates
    psum += Y^T @ M (layers 0-2: [feat, dst] transposed epilogue) or
    psum += M^T @ Y (layer 3: [dst, feat] for node-major output).
  - Epilogue per tile (layers 0-2, feat-major): +residual (DVE),
    gelu(agg + bias) in one ACT op (per-partition bias), then the next
    layer's matmul directly from the feat-major activation (no PE
    transposes anywhere), DMA into the next collective's input buffer.
"""
import sys

sys.path.insert(0, "/opt/trn_rl_repo")

import numpy as np

import concourse.bass as bass  # noqa: E402
import concourse.tile as tile  # noqa: E402
from concourse import bacc, mybir  # noqa: E402
from concourse.bass_utils import run_bass_kernel_spmd  # noqa: E402

NCORES = 8
F = 128          # feature width (all layers padded to 128)
TILES = 98       # dst tiles per core
OWN = TILES * 128            # 12544 nodes per core
NT = NCORES * OWN            # 100352 padded nodes
HTILES = TILES // 2          # dst tiles per phase (half)
HROWS = HTILES * 128         # 6272 rows per half
NWIN = 4                     # table src windows (2 cores' rows each)
NWINQ = NWIN + 1             # + window 0: self-edges, gathered from cc_in
WIN = 2 * OWN                # 25088 rows per table window (int16-addressable)
GCALL = 4096                 # idxs per dma_gather call
C_OUT = 40
YBUFS = 4
MBUFS = 3
NQUSE = 4
NPHASE = 2                   # dst-tile phases
NSW = NPHASE * NWINQ         # super-windows = (phase, window)


# --------------------------------------------------------------------------
# host-side schedule
# --------------------------------------------------------------------------

class Sched:
    """Shared (core-independent) schedule.

    runs:  list of dicts {q, t, R, s_lo (stream pos within q), first/last}
    calls: list of dicts {q, lo, n, chunks: [ {slot, segs: [
               {run_idx, e_lo, e_hi, m_idx, start, stop} ] } ]}
    MX:    total number of M tiles
    IDXC:  int16 idx columns (sum over calls of n/16)
    """


def make_schedule(R_tw):
    """R_tw: [TILES, NWINQ] run lengths. Super-window sw = phase*NWINQ + w
    covers dst tiles [phase*HTILES, (phase+1)*HTILES). Window 0 is the
    self-edge window (gathers own xw rows from cc_in, collective-free)."""
    sched = Sched()
    sched.runs = []
    qlen = [0] * NSW
    for q in range(NSW):
        p, w = divmod(q, NWINQ)
        pos = 0
        for t in range(p * HTILES, (p + 1) * HTILES):
            R = int(R_tw[t, w])
            if R == 0:
                continue
            sched.runs.append(dict(q=q, t=t, R=R, s_lo=pos, idx=len(sched.runs)))
            pos += R
        if pos % 16:  # idx stream wraps in 16s: pad the window's last run
            pad = 16 - pos % 16
            sched.runs[-1]["R"] += pad
            pos += pad
        qlen[q] = pos

    first_q, last_q = {}, {}
    for r in sched.runs:
        first_q.setdefault(r["t"], r["q"])
        last_q[r["t"]] = r["q"]
    sched.first_q, sched.last_q = first_q, last_q

    # runs of each q sorted by s_lo already
    runs_by_q = [[r for r in sched.runs if r["q"] == q] for q in range(NSW)]

    sched.calls = []
    m_idx = 0
    idxc = 0
    for q in range(NSW):
        rq = runs_by_q[q]
        pos = 0
        ri = 0
        while pos < qlen[q]:
            n = min(GCALL, qlen[q] - pos)
            call = dict(q=q, lo=pos, n=n, idx_off16=idxc, chunks=[])
            idxc += n // 16
            nch = (n + 127) // 128
            for k in range(nch):
                c_lo = pos + k * 128
                c_hi = min(pos + (k + 1) * 128, pos + n)
                segs = []
                # advance ri to first run overlapping c_lo
                while ri < len(rq) and rq[ri]["s_lo"] + rq[ri]["R"] <= c_lo:
                    ri += 1
                rj = ri
                while rj < len(rq) and rq[rj]["s_lo"] < c_hi:
                    r = rq[rj]
                    e_lo = max(r["s_lo"], c_lo) - c_lo
                    e_hi = min(r["s_lo"] + r["R"], c_hi) - c_lo
                    segs.append(dict(
                        run=r, e_lo=e_lo, e_hi=e_hi, m_idx=m_idx,
                        start=(max(r["s_lo"], c_lo) == r["s_lo"]),
                        stop=(min(r["s_lo"] + r["R"], c_hi) == r["s_lo"] + r["R"]),
                    ))
                    m_idx += 1
                    rj += 1
                call["chunks"].append(dict(slot=k, segs=segs))
            sched.calls.append(call)
            pos += n
    sched.MX = m_idx
    sched.IDXC = idxc
    sched.max_nm = max(sum(len(c["segs"]) for c in call["chunks"])
                       for call in sched.calls)
    sched.qlen = qlen
    return sched


def preprocess(x, edge_index, n_real):
    src_r = np.asarray(edge_index[0], dtype=np.int64)
    dst_r = np.asarray(edge_index[1], dtype=np.int64)

    deg = np.bincount(dst_r, minlength=NT).astype(np.float32) + 2.0
    dinv = 1.0 / np.sqrt(deg)

    # fold the self-loop term into the edge list: one self-edge per node
    # with coef = 2*dinv^2 (PyG improved=True self-loop weight).
    allv = np.arange(NT, dtype=np.int64)
    src = np.concatenate([src_r, allv])
    dst = np.concatenate([dst_r, allv])
    coef_a = np.concatenate([
        (dinv[src_r] * dinv[dst_r]).astype(np.float32),
        (2.0 * dinv * dinv).astype(np.float32),
    ])

    core = dst // OWN
    dstl_a = dst - core * OWN
    blk_a = dstl_a >> 7
    dloc_a = (dstl_a & 127).astype(np.int64)
    is_self = np.zeros(src.shape[0], dtype=bool)
    is_self[src_r.shape[0]:] = True
    w_r = np.minimum(src // WIN, NWIN - 1)

    # Per-core permutation of physical dst blocks -> logical tile slots,
    # greedily matching per-window count vectors across cores so the
    # SPMD max-over-cores run lengths carry less padding.
    cnt = np.zeros((NCORES, TILES, NWIN), np.int64)
    np.add.at(cnt, (core[~is_self], blk_a[~is_self], w_r[~is_self]), 1)
    totals = cnt.sum(axis=2)
    remaining = [list(range(TILES)) for _ in range(NCORES)]
    perm = np.zeros((NCORES, TILES), np.int64)
    for slot in range(TILES):
        b0 = max(remaining[0], key=lambda b: totals[0, b])
        remaining[0].remove(b0)
        perm[0, slot] = b0
        vec = cnt[0, b0]
        for c in range(1, NCORES):
            bb = min(remaining[c], key=lambda b: np.abs(cnt[c, b] - vec).sum())
            remaining[c].remove(bb)
            perm[c, slot] = bb
    inv_perm = np.zeros_like(perm)
    for c in range(NCORES):
        inv_perm[c, perm[c]] = np.arange(TILES)
    sched_rows = [(perm[c][:, None] * 128 + np.arange(128)).ravel()
                  for c in range(NCORES)]

    # logical dst tile of each edge
    t_a = inv_perm[core, blk_a]
    # src side: window 0 = self-edges, gathered from cc_in (own xw, static
    # address, no collective dependency; logical row order). Real edges use
    # table windows 1..4 over the AllGather output (logical order per core).
    cs_a = src // OWN
    srcl = src % OWN
    src_log = inv_perm[cs_a, srcl >> 7] * 128 + (srcl & 127)
    w_a = np.where(is_self, 0, 1 + w_r)
    idxrel_a = np.where(is_self, src_log,
                        (cs_a - 2 * w_r) * OWN + src_log).astype(np.int64)
    assert idxrel_a.max() < 32768
    phase_a = (t_a >= HTILES).astype(np.int64)
    q_a = phase_a * NWINQ + w_a

    counts = np.zeros((NCORES, TILES, NWINQ), dtype=np.int64)
    np.add.at(counts, (core, t_a, w_a), 1)
    R_tw = counts.max(axis=0).astype(np.int64)

    sched = make_schedule(R_tw)
    sched.rows_phys = sched_rows

    order = np.lexsort((idxrel_a, t_a, q_a, core))
    src_s = idxrel_a[order]
    core_s = core[order]
    t_s = t_a[order]
    q_s = q_a[order]
    dloc_s = dloc_a[order]
    coef_s = coef_a[order]

    run_pos = {(r["q"], r["t"]): r for r in sched.runs}
    # stream-global base per q
    qbase = np.cumsum([0] + sched.qlen[:-1])

    per_core = []
    for c in range(NCORES):
        sel = core_s == c
        ci, ct, cq = src_s[sel], t_s[sel], q_s[sel]
        cd, cc = dloc_s[sel], coef_s[sel]
        # flat global stream of idx / dloc / coef (padded)
        SL = int(sum(sched.qlen))
        idx_flat = np.zeros(SL, dtype=np.int16)
        dl_flat = np.zeros(SL, dtype=np.int64)
        cf_flat = np.zeros(SL, dtype=np.float32)
        key = cq * TILES + ct
        bounds = np.flatnonzero(np.r_[True, key[1:] != key[:-1], True])
        for bi in range(len(bounds) - 1):
            lo, hi = bounds[bi], bounds[bi + 1]
            r = run_pos[(int(cq[lo]), int(ct[lo]))]
            n = hi - lo
            assert n <= r["R"]
            g0 = qbase[r["q"]] + r["s_lo"]
            idx_flat[g0:g0 + n] = ci[lo:hi]
            dl_flat[g0:g0 + n] = cd[lo:hi]
            cf_flat[g0:g0 + n] = cc[lo:hi]
        # idx wrapped per call
        idx_w = np.zeros((128, sched.IDXC), dtype=np.int16)
        for call in sched.calls:
            g0 = qbase[call["q"]] + call["lo"]
            blk = idx_flat[g0:g0 + call["n"]].reshape(-1, 16).T
            o = call["idx_off16"]
            idx_w[:, o:o + call["n"] // 16] = np.tile(blk, (8, 1))
        # M tiles per segment
        M = np.zeros((sched.MX, 128, 128), dtype=np.float32)
        for call in sched.calls:
            g0 = qbase[call["q"]] + call["lo"]
            for ch in call["chunks"]:
                c_lo = g0 + ch["slot"] * 128
                for s in ch["segs"]:
                    e = np.arange(s["e_lo"], s["e_hi"])
                    gpos = c_lo + e
                    m = M[s["m_idx"]]
                    m[e, dl_flat[gpos]] = cf_flat[gpos]
        M_w = M.transpose(1, 0, 2).reshape(128, sched.MX * 128)
        per_core.append({
            "idx16": np.ascontiguousarray(idx_w),
            "M": np.ascontiguousarray(M_w),
        })

    return per_core, sched


# --------------------------------------------------------------------------
# bass program
# --------------------------------------------------------------------------

def build(sched):
    BDT = mybir.dt.bfloat16
    nc = bacc.Bacc("TRN2", target_bir_lowering=False, debug=False,
                   num_devices=NCORES, num_swdge_queues=NQUSE)

    MX, IDXC = sched.MX, sched.IDXC
    xT_in = nc.dram_tensor("xT", [128, OWN], BDT, kind="ExternalInput")
    idx16_in = nc.dram_tensor("idx16", [128, IDXC], mybir.dt.int16, kind="ExternalInput")
    m_in = nc.dram_tensor("M", [128, MX * 128], BDT, kind="ExternalInput")
    w_in = [nc.dram_tensor(f"W{l}", [128, 128], BDT, kind="ExternalInput")
            for l in range(4)]
    bcol_in = [nc.dram_tensor(f"bc{l}", [128, 1], mybir.dt.float32,
                              kind="ExternalInput") for l in range(3)]
    b3_in = nc.dram_tensor("b3t", [128, 128], mybir.dt.float32, kind="ExternalInput")
    out_dram = nc.dram_tensor("out", [OWN, 128], mybir.dt.float32, kind="ExternalOutput")

    max_call_chunks = max((c["n"] + 127) // 128 for c in sched.calls)
    # M tiles per call
    call_m0 = []
    for call in sched.calls:
        first_seg = call["chunks"][0]["segs"][0]["m_idx"]
        nm = sum(len(ch["segs"]) for ch in call["chunks"])
        call_m0.append((first_seg, nm))
    max_nm = sched.max_nm

    with tile.TileContext(nc) as tc:
        with (
            tc.tile_pool(name="persist", bufs=1) as pers,
            tc.tile_pool(name="ybuf", bufs=YBUFS) as yp,
            tc.tile_pool(name="mbuf", bufs=MBUFS) as mp,
            tc.tile_pool(name="runp", bufs=7, space="PSUM") as rp,
            tc.tile_pool(name="epip", bufs=1, space="PSUM") as ep,
            tc.tile_pool(name="etmp", bufs=4) as et,
            tc.tile_pool(name="dram", bufs=1, space="DRAM") as dp,
        ):
            # ---- persistent SBUF ----
            idx_t = pers.tile([128, IDXC], mybir.dt.int16, tag="idx")
            nc.sync.dma_start(idx_t[:], idx16_in[:])
            xT_t = pers.tile([128, OWN], BDT, tag="xT")
            nc.sync.dma_start(xT_t[:], xT_in[:])
            w_t, bc_t = [], []
            for l in range(4):
                wt = pers.tile([128, 128], BDT, tag=f"w{l}")
                nc.sync.dma_start(wt[:], w_in[l][:])
                w_t.append(wt)
            for l in range(3):
                bt = pers.tile([128, 1], mybir.dt.float32, tag=f"bc{l}")
                nc.sync.dma_start(bt[:], bcol_in[l][:])
                bc_t.append(bt)
            b3_t = pers.tile([128, 128], mybir.dt.float32, tag="b3")
            nc.sync.dma_start(b3_t[:], b3_in[:])
            agg_t = pers.tile([128, TILES * 128], mybir.dt.float32, tag="agg")
            g_t = pers.tile([128, TILES * 128], BDT, tag="g")

            # zero gather slots once (short-count gathers leave stale tails;
            # M zero rows null them unless stale bits are NaN)
            for _ in range(YBUFS):
                yz = yp.tile([128, max_call_chunks, 128], BDT, tag="y")
                nc.vector.memset(yz[:], 0.0)

            # ---- collective buffers ----
            cc_in = [dp.tile([OWN, 128], BDT, tag=f"ccin{l}",
                             name=f"ccin{l}") for l in range(4)]
            cc_out = [dp.tile([NT, 128], BDT, tag=f"ccout{i}",
                              name=f"ccout{i}", addr_space="Shared")
                      for i in range(4)]

            def fire_cc(l):
                nc.gpsimd.collective_compute(
                    "AllGather",
                    mybir.AluOpType.bypass,
                    replica_groups=[list(range(NCORES))],
                    ins=[cc_in[l][:].opt()],
                    outs=[cc_out[l][:].opt()],
                )

            def make_xw(l, t):
                lhsT = (xT_t if l == 0 else g_t)[:, t * 128:(t + 1) * 128]
                pxw = ep.tile([128, 128], mybir.dt.float32, space="PSUM", tag="pxw", bufs=1)
                nc.tensor.matmul(out=pxw[:], lhsT=lhsT, rhs=w_t[l][:],
                                 start=True, stop=True)
                xw_sb = et.tile([128, 128], BDT, tag="xwsb")
                nc.vector.tensor_copy(xw_sb[:], pxw[:])
                nc.sync.dma_start(cc_in[l][t * 128:(t + 1) * 128, :], xw_sb[:])

            # ---- layer 0 pre-phase: xw0 = x @ W0 ----
            for t in range(TILES):
                make_xw(0, t)

            gq = [0]

            def epilogue(l, t):
                agg_sl = agg_t[:, t * 128:(t + 1) * 128]
                g_sl = g_t[:, t * 128:(t + 1) * 128]
                if l == 3:
                    h = et.tile([128, 128], mybir.dt.float32, tag="h")
                    nc.vector.tensor_tensor(out=h[:], in0=agg_sl, in1=b3_t[:],
                                            op=mybir.AluOpType.add)
                    nc.sync.dma_start(out_dram[t * 128:(t + 1) * 128, :], h[:])
                    return
                if l in (1, 2):
                    nc.vector.tensor_tensor(out=agg_sl, in0=agg_sl, in1=g_sl,
                                            op=mybir.AluOpType.add)
                nc.scalar.activation(g_sl, agg_sl,
                                     mybir.ActivationFunctionType.Gelu,
                                     bias=bc_t[l][:])
                make_xw(l + 1, t)

            def do_windows(l, qset):
                """Process the given super-windows; epilogues fire inline."""
                psum_of_run = {}
                for ci, call in enumerate(sched.calls):
                    q, n = call["q"], call["n"]
                    if q not in qset:
                        continue
                    w = q % NWINQ
                    if w == 0:
                        in_rows = cc_in[l][:]
                    else:
                        in_rows = cc_out[l][(w - 1) * WIN:w * WIN, :]
                    nch = (n + 127) // 128
                    o16 = call["idx_off16"]
                    y = yp.tile([128, max_call_chunks, 128], BDT, tag="y")
                    nc.gpsimd.dma_gather(
                        out_ap=y[:, :nch, :],
                        in_ap=in_rows,
                        idxs_ap=idx_t[:, o16:o16 + n // 16],
                        num_idxs=n,
                        num_idxs_reg=n,
                        elem_size=128,
                        single_packet=False,
                        queue_num=gq[0] % NQUSE,
                    )
                    gq[0] += 1
                    m0, nm = call_m0[ci]
                    ms = mp.tile([128, max_nm * 128], BDT, tag="ms")
                    nc.scalar.dma_start(ms[:, :nm * 128],
                                        m_in[:, m0 * 128:(m0 + nm) * 128])
                    for ch in call["chunks"]:
                        k = ch["slot"]
                        for s in ch["segs"]:
                            r = s["run"]
                            rid = r["idx"]
                            if s["start"]:
                                psum_of_run[rid] = rp.tile(
                                    [128, 128], mybir.dt.float32,
                                    space="PSUM", tag="rp", name="rpt")
                            psum = psum_of_run[rid]
                            mi = s["m_idx"] - m0
                            if l == 3:
                                nc.tensor.matmul(
                                    out=psum[:],
                                    lhsT=ms[:, mi * 128:(mi + 1) * 128],
                                    rhs=y[:, k, :],
                                    start=s["start"], stop=s["stop"])
                            else:
                                nc.tensor.matmul(
                                    out=psum[:],
                                    lhsT=y[:, k, :],
                                    rhs=ms[:, mi * 128:(mi + 1) * 128],
                                    start=s["start"], stop=s["stop"])
                            if s["stop"]:
                                t = r["t"]
                                agg_sl = agg_t[:, t * 128:(t + 1) * 128]
                                if q == sched.first_q[t]:
                                    nc.scalar.copy(agg_sl, psum[:])
                                else:
                                    nc.vector.tensor_tensor(
                                        out=agg_sl, in0=agg_sl, in1=psum[:],
                                        op=mybir.AluOpType.add)
                                del psum_of_run[rid]
                                if q == sched.last_q[t]:
                                    epilogue(l, t)
                assert not psum_of_run

            for l in range(4):
                # self-edge windows first: they read cc_in (own xw) and can
                # run while the AllGather (fired just after) is in flight
                do_windows(l, {0, NWINQ})
                fire_cc(l)
                do_windows(l, {1, 2, 3, 4})
                do_windows(l, {NWINQ + 1, NWINQ + 2, NWINQ + 3, NWINQ + 4})

    nc.compile()
    return nc


# --------------------------------------------------------------------------
# public entry point
# --------------------------------------------------------------------------

def _host_inputs(x, edge_index, Ws, bs):
    import ml_dtypes
    n_real = x.shape[0]
    per_core, sched = preprocess(x, edge_index, n_real)

    xpad = np.zeros((NT, F), dtype=np.float32)
    xpad[:n_real] = np.asarray(x, dtype=np.float32)

    W3p = np.zeros((128, 128), np.float32)
    W3p[:, :C_OUT] = Ws[3]
    Wl = [np.asarray(Ws[0], np.float32), np.asarray(Ws[1], np.float32),
          np.asarray(Ws[2], np.float32), W3p]
    b3p = np.zeros(128, np.float32)
    b3p[:C_OUT] = bs[3]

    in_maps = []
    for c in range(NCORES):
        d = per_core[c]
        m = {
            "xT": xpad[c * OWN:(c + 1) * OWN][sched.rows_phys[c]].T
                  .astype(ml_dtypes.bfloat16),
            "idx16": d["idx16"],
            "M": d["M"].astype(ml_dtypes.bfloat16),
            "b3t": np.tile(b3p, (128, 1)),
        }
        for l in range(4):
            m[f"W{l}"] = Wl[l].astype(ml_dtypes.bfloat16)
        for l in range(3):
            m[f"bc{l}"] = np.asarray(bs[l], np.float32).reshape(128, 1)
        in_maps.append(m)
    return in_maps, sched


def kernel(x, edge_index, W0, b0, W1, b1, W2, b2, W3, b3):
    x = np.asarray(x)
    in_maps, sched = _host_inputs(
        x, np.asarray(edge_index), [W0, W1, W2, W3], [b0, b1, b2, b3])
    nc = build(sched)
    res = run_bass_kernel_spmd(nc, in_maps, list(range(NCORES)))
    outs = []
    for c in range(NCORES):
        o_log = res.results[c]["out"]
        o_phys = np.empty_like(o_log)
        o_phys[sched.rows_phys[c]] = o_log
        outs.append(o_phys)
    full = np.concatenate(outs, axis=0)[:x.shape[0], :C_OUT]
    return full.astype(np.float32)
